# revision 26
# baseline (speedup 1.0000x reference)
"""ActorCriticMFOSRNN Trainium2 kernel (8 NeuronCores, data-parallel over batch).

Strategy:
- Data-parallel over B=512 -> 64 batch elements per core. Params replicated.
- All recurrent compute in "transposed" layout: [H=128 partitions, cols] where
  cols = 3 heads x S time-segments x 64 batch.
- Time-segmentation: dones fire with p~0.5 per step, so T=256 is split into S
  segments run CONCURRENTLY as extra batch columns. Segments k>0 start from
  h=0 at t = k*L - W and warm up over W steps; a done inside the warmup window
  makes the output window exact. The host verifies this against the actual
  dones and picks (S, W); falls back to smaller S / serial if needed.
- bf16 matmuls (fp32 matmul is 4x slower on TRN2), fp32 PSUM accumulate.
- Gate input-side projections are fused: xp = obs @ (W_emb @ Wi) + (b_emb@Wi+bi),
  with the bias folded in via an augmented K=65 ones-row on the obs matmul.
"""
import os
import sys
import types
import numpy as np

import concourse.bass as bass
import concourse.mybir as mybir
from concourse.tile import TileContext
from concourse.vector_clock import ScopedClock, VectorClock

# ---------------------------------------------------------------------------
# Patch: this container's walrus rejects >2 sync-waits on one instruction.
# Split the TileContext tail-drain waits into per-proc nops.
# ---------------------------------------------------------------------------
def _drain_and_barrier_split(self, tick_clock, wait_clock):
    nc = self.nc
    gc = tick_clock.global_clock
    n = len(gc)
    for i in range(n):
        t = gc[i]
        if t > 0:
            vc = VectorClock([t if j == i else 0 for j in range(n)])
            nop = nc.sync.nop(nofuse=True, hint=f"tail_wait_p{i}")
            wait_clock.add_sem_waits(nop.ins, ScopedClock({None: vc}))
    nc.sync.drain()
    nc.all_engine_barrier()
    assert self.sems is not None
    popped = nc._tile_sem_poison_stack.pop()
    assert popped is self._sem_poison
    nc.clear_and_free_semaphores(list(self.sems.allocated().values()))


TileContext._drain_and_barrier = _drain_and_barrier_split

_MAXW = 1  # walrus in this container rejects >2 sync waits per instruction


def _split_waits(nc):
    """Hoist excess semaphore waits onto same-engine NoOps (walrus limit)."""
    n = 0
    for fn in nc.m.functions:
        for blk in fn.blocks:
            out = []
            for inst in blk.instructions:
                si = inst.sync_info
                if si is not None and si.on_wait and len(si.on_wait) > _MAXW:
                    waits = list(si.on_wait)
                    extra, keep = waits[:-_MAXW], waits[-_MAXW:]
                    for j in range(0, len(extra), _MAXW):
                        nop = mybir.InstNoOp(
                            name=f"waitsplit_{n}", engine=inst.engine,
                            sync_info=mybir.SyncInfo(on_wait=extra[j:j + _MAXW],
                                                     on_update=[]),
                            bass_nofuse=True)
                        out.append(nop)
                        n += 1
                    inst.sync_info = mybir.SyncInfo(on_wait=keep,
                                                    on_update=list(si.on_update))
                out.append(inst)
            blk.instructions = out
    return n

T, B, OBS, H, A = 256, 512, 64, 128, 6
NCORES = 8
BC = B // NCORES  # 64 batch per core
F32 = mybir.dt.float32
BF16 = mybir.dt.bfloat16
AF = mybir.ActivationFunctionType

_NC_CACHE = {}


# ---------------------------------------------------------------------------
# Builder
# ---------------------------------------------------------------------------
def _build(S, W):
    """Build the Bass graph for S segments with W warmup ticks."""
    L = T // S
    NT = L + W               # ticks
    FH = S * BC              # cols per head
    F = 3 * FH               # total cols

    nc = bass.Bass("TRN2", num_devices=NCORES)

    # inputs (per-core shards / replicated weights), all pre-laid-out by host
    obsT_d = nc.declare_dram_parameter("obsT", [OBS + 1, NT * FH], BF16, isOutput=False)
    m_d = nc.declare_dram_parameter("mask", [max(NT - 1, 1), F], BF16, isOutput=False)
    inj_d = nc.declare_dram_parameter("inj", [H, F], BF16, isOutput=False)
    wobs_d = nc.declare_dram_parameter("wobs", [OBS + 1, 9, H], BF16, isOutput=False)
    whh_d = nc.declare_dram_parameter("whh", [H, 9, H], BF16, isOutput=False)
    bhn_d = nc.declare_dram_parameter("bhn", [1, 3, H], BF16, isOutput=False)
    wth_d = nc.declare_dram_parameter("wth", [H, H], BF16, isOutput=False)
    wact_d = nc.declare_dram_parameter("wact", [H, A], BF16, isOutput=False)
    wcrit_d = nc.declare_dram_parameter("wcrit", [H, 1], BF16, isOutput=False)
    bth_d = nc.declare_dram_parameter("bth", [1, H], BF16, isOutput=False)
    bac_d = nc.declare_dram_parameter("bac", [1, A + 1], BF16, isOutput=False)

    ylast_d = nc.declare_dram_parameter("ylast", [H, 3 * BC], F32, isOutput=True)
    th_d = nc.declare_dram_parameter("th", [T, BC, H], F32, isOutput=True)
    ac_d = nc.declare_dram_parameter("ac", [T, BC, A + 1], F32, isOutput=True)

    NPAIR = max(S // 2, 1)   # seg-pairs for output projections (M=128)

    with TileContext(nc) as tc:
        with (
            tc.tile_pool(name="const", bufs=1) as const,
            tc.tile_pool(name="state", bufs=3) as state,
            tc.tile_pool(name="work", bufs=2) as work,
            tc.tile_pool(name="ytiles", bufs=3) as ytiles,
            tc.tile_pool(name="outs", bufs=2) as outs,
            tc.tile_pool(name="mpool", bufs=3) as mpool,
            tc.tile_pool(name="psum", bufs=1, space=bass.MemorySpace.PSUM) as psum,
            tc.tile_pool(name="psum_o", bufs=1, space=bass.MemorySpace.PSUM) as psum_o,
        ):
            # --- load constants ---
            obsT = const.tile([OBS + 1, NT, FH], BF16)
            nc.sync.dma_start(out=obsT[:], in_=obsT_d[:, :].rearrange("p (t f) -> p t f", t=NT))
            wobs = const.tile([OBS + 1, 9, H], BF16)
            nc.sync.dma_start(out=wobs[:], in_=wobs_d[:, :, :])
            whh = const.tile([H, 9, H], BF16)
            nc.sync.dma_start(out=whh[:], in_=whh_d[:, :, :])
            bhn = const.tile([1, 3, H], BF16)
            nc.sync.dma_start(out=bhn[:], in_=bhn_d[:, :, :])
            wth = const.tile([H, H], BF16)
            nc.sync.dma_start(out=wth[:], in_=wth_d[:, :])
            wact = const.tile([H, A], BF16)
            nc.sync.dma_start(out=wact[:], in_=wact_d[:, :])
            wcrit = const.tile([H, 1], BF16)
            nc.sync.dma_start(out=wcrit[:], in_=wcrit_d[:, :])
            bth = const.tile([1, H], BF16)
            nc.sync.dma_start(out=bth[:], in_=bth_d[:, :])
            bac = const.tile([1, A + 1], BF16)
            nc.sync.dma_start(out=bac[:], in_=bac_d[:, :])
            inj = const.tile([H, 3, S, BC], BF16)
            nc.sync.dma_start(out=inj[:], in_=inj_d[:, :].rearrange("p (h s b) -> p h s b", h=3, s=S))
            ones = const.tile([1, max(FH, 2 * BC)], BF16)
            nc.vector.memset(ones[:], 1.0)

            # --- initial carry ---
            c_cur = state.tile([H, 3, FH], BF16)
            if W == 0:
                nc.sync.dma_start(out=c_cur[:], in_=inj_d[:, :].rearrange("p (h f) -> p h f", h=3))
            else:
                nc.vector.memset(c_cur[:], 0.0)

            # PSUM has_written bits are cleared per-BANK by start=True, so only
            # the first matmul touching each bank per tick may use start=True;
            # later region-writers rely on overwrite-where-bit-unset.
            BANK_ELEMS = 512  # fp32 per partition per bank
            # Two PSUM tiles so the obs-side prefetch for tick i+1 can start as
            # soon as its own tile's readers (not all readers) are done:
            #   P_rn: [r_t r_a | r_c pad | n_t n_a | n_c pad]   (4 banks)
            #   P_zx: [z_t z_a | z_c x_t | x_a x_c]             (3 banks)
            RN_PAD = max(1, BANK_ELEMS // FH - 3 % (BANK_ELEMS // FH)) if FH < BANK_ELEMS else 0
            if FH == 256:
                RREG, NREG = [0, 1, 2], [4, 5, 6]
                NRN = 8
            else:
                RREG, NREG = [0, 1, 2], [3, 4, 5]
                NRN = 6
            ZREG, XREG = [0, 1, 2], [3, 4, 5]
            NZX = 6

            def _mm_opened(P, opened, region, lhsT, rhs):
                bank = (region * FH) // BANK_ELEMS
                st = bank not in opened
                opened.add(bank)
                nc.tensor.matmul(P[:, region], lhsT, rhs,
                                 start=st, stop=False, skip_group_check=True)

            def emit_obs_rn(P_rn, i):
                rhs_o = obsT[:, i]
                opened = set()
                for h in range(3):
                    _mm_opened(P_rn, opened, RREG[h], wobs[:, 3 * h + 0], rhs_o)
                    _mm_opened(P_rn, opened, NREG[h], bhn[:, h], ones[:, :FH])

            def emit_obs_zx(P_zx, i):
                rhs_o = obsT[:, i]
                opened = set()
                for h in range(3):
                    _mm_opened(P_zx, opened, ZREG[h], wobs[:, 3 * h + 1], rhs_o)
                    _mm_opened(P_zx, opened, XREG[h], wobs[:, 3 * h + 2], rhs_o)

            def emit_mask_dma(i):
                mt = mpool.tile([H, 3, FH], BF16, tag="m")
                m_ap = bass.AP(tensor=m_d, offset=i * F,
                               ap=[[0, H], [FH, 3], [1, FH]])
                nc.sync.dma_start(out=mt[:], in_=m_ap)
                return mt

            # projections are emitted one tick late so they sit BEFORE the
            # next tick's carry matmuls in PE program order (PE is in-order;
            # otherwise they delay the recurrence chain).
            def emit_proj_mms(y, j):
                """Output projection matmuls for tick j; returns the PSUM tile."""
                if j < W:
                    return None
                yv = y[:].rearrange("p h f -> p (h f)")
                po = psum_o.tile([H, NPAIR, 136], F32, tag="po")
                thp = po[:, :, 0:H]
                acp = po[:, :, H:H + A + 1]
                first = [True]

                def pmm(out_ap, lhsT, rhs):
                    nc.tensor.matmul(out_ap, lhsT, rhs, start=first[0],
                                     stop=False, skip_group_check=True)
                    first[0] = False

                for p in range(NPAIR):
                    sl = slice(p * 128, p * 128 + 128) if S > 1 else slice(0, BC)
                    mdim = 128 if S > 1 else BC
                    pmm(thp[:mdim, p], ones[:, :mdim], bth[:])
                    pmm(thp[:mdim, p], yv[:, sl], wth[:])
                    pmm(acp[:mdim, p], ones[:, :mdim], bac[:])
                    pmm(acp[:mdim, p, 0:A], yv[:, FH + sl.start:FH + sl.stop], wact[:])
                    pmm(acp[:mdim, p, A:A + 1], yv[:, 2 * FH + sl.start:2 * FH + sl.stop], wcrit[:])
                return po

            def emit_proj_tail(po, j):
                if po is None:
                    return
                thp = po[:, :, 0:H]
                acp = po[:, :, H:H + A + 1]
                acsb = outs.tile([H, NPAIR, A + 1], F32, tag="acsb")
                nc.vector.tensor_copy(acsb[:], acp[:])
                thsb = outs.tile([H, NPAIR, H], F32, tag="thsb")
                nc.scalar.activation(thsb[:], thp[:], AF.Sigmoid)
                for k in range(S):
                    t_idx = k * L - W + j
                    pr, half = divmod(k, 2) if S > 1 else (0, 0)
                    rows = slice(half * BC, half * BC + BC)
                    nc.gpsimd.dma_start(out=th_d[t_idx], in_=thsb[rows, pr])
                    nc.gpsimd.dma_start(out=ac_d[t_idx], in_=acsb[rows, pr])

            # prologue: obs-side work for tick 0
            P_rn = psum.tile([H, NRN, FH], F32, tag="Prn")
            P_zx = psum.tile([H, NZX, FH], F32, tag="Pzx")
            emit_obs_rn(P_rn, 0)
            emit_obs_zx(P_zx, 0)
            mt = emit_mask_dma(0) if NT > 1 else None
            y_prev = None

            for i in range(NT):
                # ---- r-gate carry matmuls first so sigmoid(r) starts early
                for h in range(3):
                    nc.tensor.matmul(P_rn[:, RREG[h]], whh[:, 3 * h + 0], c_cur[:, h],
                                     start=False, stop=True, skip_group_check=True)
                r_bf = work.tile([H, 3, FH], BF16, tag="rbf")
                nc.scalar.activation(r_bf[:], P_rn[:, RREG[0]:RREG[0] + 3], AF.Sigmoid)
                for h in range(3):
                    nc.tensor.matmul(P_rn[:, NREG[h]], whh[:, 3 * h + 2], c_cur[:, h],
                                     start=False, stop=True, skip_group_check=True)
                # xpn extraction on DVE (its inputs were ready last tick) and
                # rq emitted BEFORE proj/z so their sem waits don't get aligned
                # behind lower-urgency PE work
                xpn = work.tile([H, 3, FH], BF16, tag="xpn")
                nc.vector.tensor_copy(xpn[:], P_zx[:, XREG[0]:XREG[0] + 3])
                rq = work.tile([H, 3, FH], BF16, tag="rq")
                nc.vector.tensor_mul(rq[:], r_bf[:], P_rn[:, NREG[0]:NREG[0] + 3])

                # deferred projections of the previous tick (PE fill-in work,
                # placed before the z matmuls which aren't needed until late)
                po_prev = None
                if y_prev is not None:
                    po_prev = emit_proj_mms(y_prev, i - 1)

                for h in range(3):
                    nc.tensor.matmul(P_zx[:, ZREG[h]], whh[:, 3 * h + 1], c_cur[:, h],
                                     start=False, stop=True, skip_group_check=True)

                # r/n prefetch can start once sigmoid(r) + rq released P_rn
                Prn_next = Pzx_next = None
                if i + 1 < NT:
                    Prn_next = psum.tile([H, NRN, FH], F32, tag="Prn")
                    emit_obs_rn(Prn_next, i + 1)
                    mt_next = emit_mask_dma(i + 1) if i + 1 < NT - 1 else None

                z_bf = work.tile([H, 3, FH], BF16, tag="zbf")
                nc.scalar.activation(z_bf[:], P_zx[:, ZREG[0]:ZREG[0] + 3], AF.Sigmoid)
                zp = work.tile([H, 3, FH], BF16, tag="zp")
                nc.scalar.activation(zp[:], P_zx[:, ZREG[0]:ZREG[0] + 3], AF.Sigmoid,
                                     scale=-1.0)

                # z/xpn prefetch after sigmoid(z) + xpn extraction release P_zx
                if i + 1 < NT:
                    Pzx_next = psum.tile([H, NZX, FH], F32, tag="Pzx")
                    emit_obs_zx(Pzx_next, i + 1)

                # ---- rest of the chain:  y = (1-z)*n + z*c,  with z*c and
                # (1-z) computed off the critical path during tanh
                s = work.tile([H, 3, FH], BF16, tag="s")
                nc.vector.tensor_add(s[:], rq[:], xpn[:])
                zc = work.tile([H, 3, FH], BF16, tag="zc")
                nc.vector.tensor_mul(zc[:], z_bf[:], c_cur[:])
                n_bf = work.tile([H, 3, FH], BF16, tag="n")
                nc.scalar.activation(n_bf[:], s[:], AF.Tanh)
                emit_proj_tail(po_prev, i - 1)
                u = work.tile([H, 3, FH], BF16, tag="u")
                nc.vector.tensor_mul(u[:], n_bf[:], zp[:])
                y = ytiles.tile([H, 3, FH], BF16, tag="y")
                nc.vector.tensor_add(y[:], u[:], zc[:])

                # ---- next carry
                if i < NT - 1:
                    c_new = state.tile([H, 3, FH], BF16, tag="c")
                    nc.vector.tensor_mul(c_new[:], y[:], mt[:])
                    mt = mt_next
                    if i == W - 1 and W > 0:
                        c2 = state.tile([H, 3, FH], BF16, tag="c2")
                        nc.vector.tensor_add(c2[:], c_new[:], inj[:].rearrange("p h s b -> p h (s b)"))
                        c_cur = c2
                    else:
                        c_cur = c_new

                y_prev = y

                # ---- final hidden state
                if i == NT - 1:
                    emit_proj_tail(emit_proj_mms(y, i), i)
                    yf = outs.tile([H, 3, BC], F32, tag="yf")
                    nc.vector.tensor_copy(yf[:], y[:, :, (S - 1) * BC: S * BC])
                    nc.sync.dma_start(out=ylast_d[:, :], in_=yf[:].rearrange("p h b -> p (h b)"))

                P_rn, P_zx = Prn_next, Pzx_next
    _split_waits(nc)
    return nc


# ---------------------------------------------------------------------------
# Host-side prep
# ---------------------------------------------------------------------------
def _choose_sw(dones):
    """Pick (S, W) segmentation plus the set of batch columns whose
    outputs the host must recompute exactly (no done inside a warmup
    window). Falls back to serial if the bad set would be large."""
    best = (1, 0, np.zeros(0, np.int64))
    for S, W in [(4, 16), (4, 32), (2, 32), (2, 64)]:
        L = T // S
        bad = np.zeros(B, bool)
        for k in range(1, S):
            bad |= ~dones[max(k * L - W, 0): k * L + 1].any(axis=0)
        badcols = np.where(bad)[0]
        if len(badcols) <= B // 16:
            return S, W, badcols
    return best


def _prep_core(c, S, W, obs, dones, hidden, weights):
    L = T // S
    NT = L + W
    FH = S * BC
    F = 3 * FH
    b0 = c * BC

    obs_c = obs[:, b0:b0 + BC, :]
    opad = np.zeros((W + T, BC, OBS), np.float32)
    opad[W:] = obs_c
    idx = np.arange(NT)[:, None] + (np.arange(S) * L)[None, :]
    og = opad[idx]                                   # [NT, S, BC, OBS]
    obsT = og.transpose(3, 0, 1, 2).reshape(OBS, NT * FH)
    obsT65 = np.concatenate([obsT, np.ones((1, NT * FH), np.float32)], 0)

    d_c = dones[:, b0:b0 + BC].astype(np.float32)
    dpad = np.ones((W + T, BC), np.float32)
    dpad[W:] = d_c
    if NT > 1:
        midx = np.arange(1, NT)[:, None] + (np.arange(S) * L)[None, :]
        M = 1.0 - dpad[midx]                         # [NT-1, S, BC]
        if W > 0:
            M[W - 1, 0, :] = 0.0
        m_full = np.broadcast_to(M[:, None, :, :], (NT - 1, 3, S, BC)).reshape(NT - 1, F)
    else:
        m_full = np.zeros((1, F), np.float32)

    h0 = hidden[b0:b0 + BC]                          # [BC, 384]
    keep0 = 1.0 - d_c[0]                             # [BC]
    injv = np.zeros((H, 3, S, BC), np.float32)
    for h in range(3):
        injv[:, h, 0, :] = (h0[:, h * H:(h + 1) * H] * keep0[:, None]).T
    inj = injv.reshape(H, F)

    mp = {"obsT": _bf(obsT65), "mask": _bf(m_full), "inj": _bf(inj)}
    mp.update(weights)
    return mp


def _bf(x):
    import jax.numpy as jnp
    return np.asarray(jnp.asarray(x, dtype=jnp.bfloat16))


def _prep_weights(inp):
    wobs = np.zeros((OBS + 1, 9, H), np.float32)
    whh = np.zeros((H, 9, H), np.float32)
    bhn_l = np.zeros((1, 3, H), np.float32)
    for h, nm in enumerate(("t", "a", "c")):
        wc = inp[f"W_emb_{nm}"] @ inp[f"Wi_{nm}"]                      # [64, 384]
        bc = inp[f"b_emb_{nm}"] @ inp[f"Wi_{nm}"] + inp[f"bi_{nm}"]    # [384]
        for g in range(3):
            wobs[:OBS, 3 * h + g] = wc[:, g * H:(g + 1) * H]
            wobs[OBS, 3 * h + g] = bc[g * H:(g + 1) * H]
        whh[:, 3 * h + 0] = inp[f"Whrz_{nm}"][:, :H]
        whh[:, 3 * h + 1] = inp[f"Whrz_{nm}"][:, H:]
        whh[:, 3 * h + 2] = inp[f"Whn_{nm}"]
        bhn_l[0, h] = inp[f"bhn_{nm}"]
    bac = np.concatenate([inp["b_act"], inp["b_crit"]])[None, :]
    return {
        "wobs": _bf(wobs), "whh": _bf(whh), "bhn": _bf(bhn_l),
        "wth": _bf(inp["W_th"]), "wact": _bf(inp["W_act"]),
        "wcrit": _bf(inp["W_crit"]), "bth": _bf(inp["b_th"][None, :]),
        "bac": _bf(bac),
    }


def _host_exact_cols(inp, cols):
    """f32 reference recompute of the GRU stack for a subset of batch cols."""
    obs = inp["obs"][:, cols, :].astype(np.float32)        # [T, nb, OBS]
    h = {nm: inp["hidden"][cols, i * H:(i + 1) * H].astype(np.float32)
         for i, nm in enumerate(("t", "a", "c"))}
    d = inp["dones"][:, cols].astype(np.float32)           # [T, nb]
    ys = {}
    for nm in ("t", "a", "c"):
        emb = obs @ inp[f"W_emb_{nm}"] + inp[f"b_emb_{nm}"]
        xp = emb @ inp[f"Wi_{nm}"] + inp[f"bi_{nm}"]       # [T, nb, 3H]
        Whrz, Whn, bhn = inp[f"Whrz_{nm}"], inp[f"Whn_{nm}"], inp[f"bhn_{nm}"]
        hh = h[nm]
        y = np.zeros((T, len(cols), H), np.float32)
        for t in range(T):
            hh = hh * (1.0 - d[t])[:, None]
            hrz = hh @ Whrz
            r = 1.0 / (1.0 + np.exp(-(xp[t, :, :H] + hrz[:, :H])))
            z = 1.0 / (1.0 + np.exp(-(xp[t, :, H:2 * H] + hrz[:, H:])))
            n = np.tanh(xp[t, :, 2 * H:] + r * (hh @ Whn + bhn))
            hh = (1.0 - z) * n + z * hh
            y[t] = hh
        ys[nm] = y
        h[nm] = hh
    hid = np.concatenate([h["t"], h["a"], h["c"]], -1)
    act = ys["a"] @ inp["W_act"] + inp["b_act"]
    crit = (ys["c"] @ inp["W_crit"] + inp["b_crit"])[..., 0]
    th = 1.0 / (1.0 + np.exp(-(ys["t"] @ inp["W_th"] + inp["b_th"])))
    return hid, act, crit, th


# ---------------------------------------------------------------------------
# Entry
# ---------------------------------------------------------------------------
def _run(inputs, trace=False):
    import concourse.bass_utils as bass_utils
    if trace:
        import antenv
        from trn_agent_boot.trn_boot import _ntff_profile_via_ctypes
        if "antenv.axon_hooks" not in sys.modules:
            _m = types.ModuleType("antenv.axon_hooks")
            _h = _ntff_profile_via_ctypes('/opt/axon/libaxon_pjrt.so')
            _m.get_axon_ntff_profile_hook = lambda: _h
            _m.set_axon_ntff_profile_hook = lambda h: None
            sys.modules["antenv.axon_hooks"] = _m
            antenv.axon_hooks = _m
        bass_utils.upload_artifacts = lambda d: d

    inputs = {k: np.asarray(v) for k, v in inputs.items()}
    obs = inputs["obs"].astype(np.float32)
    dones = inputs["dones"].astype(bool)
    hidden = inputs["hidden"].astype(np.float32)

    S, W, badcols = _choose_sw(dones)
    if (S, W) not in _NC_CACHE:
        _NC_CACHE[(S, W)] = _build(S, W)
    nc = _NC_CACHE[(S, W)]

    weights = _prep_weights(inputs)
    in_maps = [_prep_core(c, S, W, obs, dones, hidden, weights)
               for c in range(NCORES)]
    res = bass_utils.run_bass_kernel_spmd(nc, in_maps, core_ids=list(range(NCORES)),
                                          trace=trace)

    hidden_out = np.zeros((B, 3 * H), np.float32)
    actor = np.zeros((T, B, A), np.float32)
    critic = np.zeros((T, B), np.float32)
    th = np.zeros((T, B, H), np.float32)
    for c in range(NCORES):
        b0 = c * BC
        r = res.results[c]
        hidden_out[b0:b0 + BC] = r["ylast"].reshape(H, 3, BC).transpose(2, 1, 0).reshape(BC, 3 * H)
        th[:, b0:b0 + BC, :] = r["th"]
        actor[:, b0:b0 + BC, :] = r["ac"][:, :, :A]
        critic[:, b0:b0 + BC] = r["ac"][:, :, A]

    if len(badcols):
        # exact host recompute for columns the segmentation cannot cover
        hid_p, act_p, crit_p, th_p = _host_exact_cols(inputs, badcols)
        hidden_out[badcols] = hid_p
        actor[:, badcols, :] = act_p
        critic[:, badcols] = crit_p
        th[:, badcols, :] = th_p
    return (hidden_out, actor, critic, th), res


def kernel(**inputs):
    out, _ = _run(inputs, trace=False)
    return out


# revision 28
# speedup vs baseline: 1.2899x; 1.2899x over previous
"""ActorCriticMFOSRNN Trainium2 kernel (8 NeuronCores, data-parallel over batch).

Strategy:
- Data-parallel over B=512 -> 64 batch elements per core. Params replicated.
- All recurrent compute in "transposed" layout: [H=128 partitions, cols] where
  cols = 3 heads x S time-segments x 64 batch.
- Time-segmentation: dones fire with p~0.5 per step, so T=256 is split into S
  segments run CONCURRENTLY as extra batch columns. Segments k>0 start from
  h=0 at t = k*L - W and warm up over W steps; a done inside the warmup window
  makes the output window exact. The host verifies this against the actual
  dones and picks (S, W); falls back to smaller S / serial if needed.
- bf16 matmuls (fp32 matmul is 4x slower on TRN2), fp32 PSUM accumulate.
- Gate input-side projections are fused: xp = obs @ (W_emb @ Wi) + (b_emb@Wi+bi),
  with the bias folded in via an augmented K=65 ones-row on the obs matmul.
"""
import os
import sys
import types
import numpy as np

import concourse.bass as bass
import concourse.mybir as mybir
from concourse.tile import TileContext
from concourse.vector_clock import ScopedClock, VectorClock

# ---------------------------------------------------------------------------
# Patch: this container's walrus rejects >2 sync-waits on one instruction.
# Split the TileContext tail-drain waits into per-proc nops.
# ---------------------------------------------------------------------------
def _drain_and_barrier_split(self, tick_clock, wait_clock):
    nc = self.nc
    gc = tick_clock.global_clock
    n = len(gc)
    for i in range(n):
        t = gc[i]
        if t > 0:
            vc = VectorClock([t if j == i else 0 for j in range(n)])
            nop = nc.sync.nop(nofuse=True, hint=f"tail_wait_p{i}")
            wait_clock.add_sem_waits(nop.ins, ScopedClock({None: vc}))
    nc.sync.drain()
    nc.all_engine_barrier()
    assert self.sems is not None
    popped = nc._tile_sem_poison_stack.pop()
    assert popped is self._sem_poison
    nc.clear_and_free_semaphores(list(self.sems.allocated().values()))


TileContext._drain_and_barrier = _drain_and_barrier_split

_MAXW = 1  # walrus in this container rejects >2 sync waits per instruction


def _split_waits(nc):
    """Hoist excess semaphore waits onto same-engine NoOps (walrus limit)."""
    n = 0
    for fn in nc.m.functions:
        for blk in fn.blocks:
            out = []
            for inst in blk.instructions:
                si = inst.sync_info
                if si is not None and si.on_wait and len(si.on_wait) > _MAXW:
                    waits = list(si.on_wait)
                    extra, keep = waits[:-_MAXW], waits[-_MAXW:]
                    for j in range(0, len(extra), _MAXW):
                        nop = mybir.InstNoOp(
                            name=f"waitsplit_{n}", engine=inst.engine,
                            sync_info=mybir.SyncInfo(on_wait=extra[j:j + _MAXW],
                                                     on_update=[]),
                            bass_nofuse=True)
                        out.append(nop)
                        n += 1
                    inst.sync_info = mybir.SyncInfo(on_wait=keep,
                                                    on_update=list(si.on_update))
                out.append(inst)
            blk.instructions = out
    return n

T, B, OBS, H, A = 256, 512, 64, 128, 6
NCORES = 8
BC = B // NCORES  # 64 batch per core
F32 = mybir.dt.float32
BF16 = mybir.dt.bfloat16
AF = mybir.ActivationFunctionType

_NC_CACHE = {}


# ---------------------------------------------------------------------------
# Builder
# ---------------------------------------------------------------------------
def _build(S, W):
    """Build the Bass graph for S segments with W warmup ticks."""
    L = T // S
    NT = L + W               # ticks
    FH = S * BC              # cols per head
    F = 3 * FH               # total cols

    nc = bass.Bass("TRN2", num_devices=NCORES)

    # inputs (per-core shards / replicated weights), all pre-laid-out by host
    obsT_d = nc.declare_dram_parameter("obsT", [OBS + 1, NT * FH], BF16, isOutput=False)
    m_d = nc.declare_dram_parameter("mask", [max(NT - 1, 1), F], BF16, isOutput=False)
    inj_d = nc.declare_dram_parameter("inj", [H, F], BF16, isOutput=False)
    wobs_d = nc.declare_dram_parameter("wobs", [OBS + 1, 9, H], BF16, isOutput=False)
    whh_d = nc.declare_dram_parameter("whh", [H, 9, H], BF16, isOutput=False)
    bhn_d = nc.declare_dram_parameter("bhn", [1, 3, H], BF16, isOutput=False)
    wth_d = nc.declare_dram_parameter("wth", [H, H], BF16, isOutput=False)
    wact_d = nc.declare_dram_parameter("wact", [H, A], BF16, isOutput=False)
    wcrit_d = nc.declare_dram_parameter("wcrit", [H, 1], BF16, isOutput=False)
    bth_d = nc.declare_dram_parameter("bth", [1, H], BF16, isOutput=False)
    bac_d = nc.declare_dram_parameter("bac", [1, A + 1], BF16, isOutput=False)

    ylast_d = nc.declare_dram_parameter("ylast", [H, 3 * BC], F32, isOutput=True)
    th_d = nc.declare_dram_parameter("th", [T, BC, H], F32, isOutput=True)
    ac_d = nc.declare_dram_parameter("ac", [T, BC, A + 1], F32, isOutput=True)

    NPAIR = max(S // 2, 1)   # seg-pairs for output projections (M=128)

    with TileContext(nc) as tc:
        with (
            tc.tile_pool(name="const", bufs=1) as const,
            tc.tile_pool(name="state", bufs=3) as state,
            tc.tile_pool(name="work", bufs=2) as work,
            tc.tile_pool(name="ytiles", bufs=3) as ytiles,
            tc.tile_pool(name="outs", bufs=2) as outs,
            tc.tile_pool(name="mpool", bufs=3) as mpool,
            tc.tile_pool(name="psum", bufs=1, space=bass.MemorySpace.PSUM) as psum,
            tc.tile_pool(name="psum_o", bufs=1, space=bass.MemorySpace.PSUM) as psum_o,
        ):
            # --- load constants ---
            obsT = const.tile([OBS + 1, NT, FH], BF16)
            nc.sync.dma_start(out=obsT[:], in_=obsT_d[:, :].rearrange("p (t f) -> p t f", t=NT))
            wobs = const.tile([OBS + 1, 9, H], BF16)
            nc.sync.dma_start(out=wobs[:], in_=wobs_d[:, :, :])
            whh = const.tile([H, 9, H], BF16)
            nc.sync.dma_start(out=whh[:], in_=whh_d[:, :, :])
            bhn = const.tile([1, 3, H], BF16)
            nc.sync.dma_start(out=bhn[:], in_=bhn_d[:, :, :])
            wth = const.tile([H, H], BF16)
            nc.sync.dma_start(out=wth[:], in_=wth_d[:, :])
            wact = const.tile([H, A], BF16)
            nc.sync.dma_start(out=wact[:], in_=wact_d[:, :])
            wcrit = const.tile([H, 1], BF16)
            nc.sync.dma_start(out=wcrit[:], in_=wcrit_d[:, :])
            bth = const.tile([1, H], BF16)
            nc.sync.dma_start(out=bth[:], in_=bth_d[:, :])
            bac = const.tile([1, A + 1], BF16)
            nc.sync.dma_start(out=bac[:], in_=bac_d[:, :])
            inj = const.tile([H, 3, S, BC], BF16)
            nc.sync.dma_start(out=inj[:], in_=inj_d[:, :].rearrange("p (h s b) -> p h s b", h=3, s=S))
            ones = const.tile([1, max(FH, 2 * BC)], BF16)
            nc.vector.memset(ones[:], 1.0)

            # --- initial carry ---
            c_cur = state.tile([H, 3, FH], BF16)
            if W == 0:
                nc.sync.dma_start(out=c_cur[:], in_=inj_d[:, :].rearrange("p (h f) -> p h f", h=3))
            else:
                nc.vector.memset(c_cur[:], 0.0)

            # PSUM has_written bits are cleared per-BANK by start=True, so only
            # the first matmul touching each bank per tick may use start=True;
            # later region-writers rely on overwrite-where-bit-unset.
            BANK_ELEMS = 512  # fp32 per partition per bank
            # Two PSUM tiles so the obs-side prefetch for tick i+1 can start as
            # soon as its own tile's readers (not all readers) are done:
            #   P_rn: [r_t r_a | r_c pad | n_t n_a | n_c pad]   (4 banks)
            #   P_zx: [z_t z_a | z_c x_t | x_a x_c]             (3 banks)
            RN_PAD = max(1, BANK_ELEMS // FH - 3 % (BANK_ELEMS // FH)) if FH < BANK_ELEMS else 0
            if FH == 256:
                RREG, NREG = [0, 1, 2], [4, 5, 6]
                NRN = 8
            else:
                RREG, NREG = [0, 1, 2], [3, 4, 5]
                NRN = 6
            ZREG, XREG = [0, 1, 2], [3, 4, 5]
            NZX = 6

            def _mm_opened(P, opened, region, lhsT, rhs):
                bank = (region * FH) // BANK_ELEMS
                st = bank not in opened
                opened.add(bank)
                nc.tensor.matmul(P[:, region], lhsT, rhs,
                                 start=st, stop=False, skip_group_check=True)

            def emit_obs_rn(P_rn, i):
                rhs_o = obsT[:, i]
                opened = set()
                for h in range(3):
                    _mm_opened(P_rn, opened, RREG[h], wobs[:, 3 * h + 0], rhs_o)
                    _mm_opened(P_rn, opened, NREG[h], bhn[:, h], ones[:, :FH])

            def emit_obs_zx(P_zx, i):
                rhs_o = obsT[:, i]
                opened = set()
                for h in range(3):
                    _mm_opened(P_zx, opened, ZREG[h], wobs[:, 3 * h + 1], rhs_o)
                    _mm_opened(P_zx, opened, XREG[h], wobs[:, 3 * h + 2], rhs_o)

            def emit_mask_dma(i):
                mt = mpool.tile([H, 3, FH], BF16, tag="m")
                m_ap = bass.AP(tensor=m_d, offset=i * F,
                               ap=[[0, H], [FH, 3], [1, FH]])
                nc.sync.dma_start(out=mt[:], in_=m_ap)
                return mt

            # projections are emitted one tick late so they sit BEFORE the
            # next tick's carry matmuls in PE program order (PE is in-order;
            # otherwise they delay the recurrence chain).
            def emit_proj_mms(y, j):
                """Output projection matmuls for tick j; returns the PSUM tile."""
                if j < W:
                    return None
                yv = y[:].rearrange("p h f -> p (h f)")
                po = psum_o.tile([H, NPAIR, 136], F32, tag="po")
                thp = po[:, :, 0:H]
                acp = po[:, :, H:H + A + 1]
                first = [True]

                def pmm(out_ap, lhsT, rhs):
                    nc.tensor.matmul(out_ap, lhsT, rhs, start=first[0],
                                     stop=False, skip_group_check=True)
                    first[0] = False

                for p in range(NPAIR):
                    sl = slice(p * 128, p * 128 + 128) if S > 1 else slice(0, BC)
                    mdim = 128 if S > 1 else BC
                    pmm(thp[:mdim, p], ones[:, :mdim], bth[:])
                    pmm(thp[:mdim, p], yv[:, sl], wth[:])
                    pmm(acp[:mdim, p], ones[:, :mdim], bac[:])
                    pmm(acp[:mdim, p, 0:A], yv[:, FH + sl.start:FH + sl.stop], wact[:])
                    pmm(acp[:mdim, p, A:A + 1], yv[:, 2 * FH + sl.start:2 * FH + sl.stop], wcrit[:])
                return po

            def emit_proj_tail(po, j):
                if po is None:
                    return
                thp = po[:, :, 0:H]
                acp = po[:, :, H:H + A + 1]
                acsb = outs.tile([H, NPAIR, A + 1], F32, tag="acsb")
                nc.vector.tensor_copy(acsb[:], acp[:])
                thsb = outs.tile([H, NPAIR, H], F32, tag="thsb")
                nc.scalar.activation(thsb[:], thp[:], AF.Sigmoid)
                for k in range(S):
                    t_idx = k * L - W + j
                    pr, half = divmod(k, 2) if S > 1 else (0, 0)
                    rows = slice(half * BC, half * BC + BC)
                    nc.gpsimd.dma_start(out=th_d[t_idx], in_=thsb[rows, pr])
                    nc.gpsimd.dma_start(out=ac_d[t_idx], in_=acsb[rows, pr])

            # prologue: obs-side work for tick 0
            P_rn = psum.tile([H, NRN, FH], F32, tag="Prn")
            P_zx = psum.tile([H, NZX, FH], F32, tag="Pzx")
            emit_obs_rn(P_rn, 0)
            emit_obs_zx(P_zx, 0)
            mt = emit_mask_dma(0) if NT > 1 else None
            y_prev = None

            for i in range(NT):
                # ---- r-gate carry matmuls first so sigmoid(r) starts early
                for h in range(3):
                    nc.tensor.matmul(P_rn[:, RREG[h]], whh[:, 3 * h + 0], c_cur[:, h],
                                     start=False, stop=True, skip_group_check=True)
                r_bf = work.tile([H, 3, FH], BF16, tag="rbf")
                nc.scalar.activation(r_bf[:], P_rn[:, RREG[0]:RREG[0] + 3], AF.Sigmoid)
                for h in range(3):
                    nc.tensor.matmul(P_rn[:, NREG[h]], whh[:, 3 * h + 2], c_cur[:, h],
                                     start=False, stop=True, skip_group_check=True)
                # xpn extraction on DVE (its inputs were ready last tick) and
                # rq emitted BEFORE proj/z so their sem waits don't get aligned
                # behind lower-urgency PE work
                xpn = work.tile([H, 3, FH], BF16, tag="xpn")
                nc.vector.tensor_copy(xpn[:], P_zx[:, XREG[0]:XREG[0] + 3])
                rq = work.tile([H, 3, FH], BF16, tag="rq")
                nc.vector.tensor_mul(rq[:], r_bf[:], P_rn[:, NREG[0]:NREG[0] + 3])

                # deferred projections of the previous tick (PE fill-in work,
                # placed before the z matmuls which aren't needed until late)
                po_prev = None
                if y_prev is not None:
                    po_prev = emit_proj_mms(y_prev, i - 1)

                for h in range(3):
                    nc.tensor.matmul(P_zx[:, ZREG[h]], whh[:, 3 * h + 1], c_cur[:, h],
                                     start=False, stop=True, skip_group_check=True)

                # r/n prefetch can start once sigmoid(r) + rq released P_rn
                Prn_next = Pzx_next = None
                if i + 1 < NT:
                    Prn_next = psum.tile([H, NRN, FH], F32, tag="Prn")
                    emit_obs_rn(Prn_next, i + 1)
                    mt_next = emit_mask_dma(i + 1) if i + 1 < NT - 1 else None

                z_bf = work.tile([H, 3, FH], BF16, tag="zbf")
                nc.scalar.activation(z_bf[:], P_zx[:, ZREG[0]:ZREG[0] + 3], AF.Sigmoid)

                # z/xpn prefetch after sigmoid(z) + xpn extraction release P_zx
                if i + 1 < NT:
                    Pzx_next = psum.tile([H, NZX, FH], F32, tag="Pzx")
                    emit_obs_zx(Pzx_next, i + 1)

                # ---- rest of the chain
                s = work.tile([H, 3, FH], BF16, tag="s")
                nc.vector.tensor_add(s[:], rq[:], xpn[:])
                n_bf = work.tile([H, 3, FH], BF16, tag="n")
                nc.scalar.activation(n_bf[:], s[:], AF.Tanh)
                emit_proj_tail(po_prev, i - 1)
                d = work.tile([H, 3, FH], BF16, tag="d")
                nc.vector.tensor_sub(d[:], c_cur[:], n_bf[:])
                e = work.tile([H, 3, FH], BF16, tag="e")
                nc.vector.tensor_mul(e[:], z_bf[:], d[:])
                y = ytiles.tile([H, 3, FH], BF16, tag="y")
                nc.vector.tensor_add(y[:], n_bf[:], e[:])

                # ---- next carry
                if i < NT - 1:
                    c_new = state.tile([H, 3, FH], BF16, tag="c")
                    nc.vector.tensor_mul(c_new[:], y[:], mt[:])
                    mt = mt_next
                    if i == W - 1 and W > 0:
                        c2 = state.tile([H, 3, FH], BF16, tag="c2")
                        nc.vector.tensor_add(c2[:], c_new[:], inj[:].rearrange("p h s b -> p h (s b)"))
                        c_cur = c2
                    else:
                        c_cur = c_new

                y_prev = y

                # ---- final hidden state
                if i == NT - 1:
                    emit_proj_tail(emit_proj_mms(y, i), i)
                    yf = outs.tile([H, 3, BC], F32, tag="yf")
                    nc.vector.tensor_copy(yf[:], y[:, :, (S - 1) * BC: S * BC])
                    nc.sync.dma_start(out=ylast_d[:, :], in_=yf[:].rearrange("p h b -> p (h b)"))

                P_rn, P_zx = Prn_next, Pzx_next
    _split_waits(nc)
    return nc


# ---------------------------------------------------------------------------
# Host-side prep
# ---------------------------------------------------------------------------
def _choose_sw(dones):
    """Pick (S, W) segmentation plus the set of batch columns whose
    outputs the host must recompute exactly (no done inside a warmup
    window). Falls back to serial if the bad set would be large."""
    best = (1, 0, np.zeros(0, np.int64))
    for S, W in [(4, 12), (4, 16), (4, 32), (2, 32), (2, 64)]:
        L = T // S
        bad = np.zeros(B, bool)
        for k in range(1, S):
            bad |= ~dones[max(k * L - W, 0): k * L + 1].any(axis=0)
        badcols = np.where(bad)[0]
        if len(badcols) <= B // 16:
            return S, W, badcols
    return best


def _prep_core(c, S, W, obs, dones, hidden, weights):
    L = T // S
    NT = L + W
    FH = S * BC
    F = 3 * FH
    b0 = c * BC

    obs_c = obs[:, b0:b0 + BC, :]
    opad = np.zeros((W + T, BC, OBS), np.float32)
    opad[W:] = obs_c
    idx = np.arange(NT)[:, None] + (np.arange(S) * L)[None, :]
    og = opad[idx]                                   # [NT, S, BC, OBS]
    obsT = og.transpose(3, 0, 1, 2).reshape(OBS, NT * FH)
    obsT65 = np.concatenate([obsT, np.ones((1, NT * FH), np.float32)], 0)

    d_c = dones[:, b0:b0 + BC].astype(np.float32)
    dpad = np.ones((W + T, BC), np.float32)
    dpad[W:] = d_c
    if NT > 1:
        midx = np.arange(1, NT)[:, None] + (np.arange(S) * L)[None, :]
        M = 1.0 - dpad[midx]                         # [NT-1, S, BC]
        if W > 0:
            M[W - 1, 0, :] = 0.0
        m_full = np.broadcast_to(M[:, None, :, :], (NT - 1, 3, S, BC)).reshape(NT - 1, F)
    else:
        m_full = np.zeros((1, F), np.float32)

    h0 = hidden[b0:b0 + BC]                          # [BC, 384]
    keep0 = 1.0 - d_c[0]                             # [BC]
    injv = np.zeros((H, 3, S, BC), np.float32)
    for h in range(3):
        injv[:, h, 0, :] = (h0[:, h * H:(h + 1) * H] * keep0[:, None]).T
    inj = injv.reshape(H, F)

    mp = {"obsT": _bf(obsT65), "mask": _bf(m_full), "inj": _bf(inj)}
    mp.update(weights)
    return mp


def _bf(x):
    import jax.numpy as jnp
    return np.asarray(jnp.asarray(x, dtype=jnp.bfloat16))


def _prep_weights(inp):
    wobs = np.zeros((OBS + 1, 9, H), np.float32)
    whh = np.zeros((H, 9, H), np.float32)
    bhn_l = np.zeros((1, 3, H), np.float32)
    for h, nm in enumerate(("t", "a", "c")):
        wc = inp[f"W_emb_{nm}"] @ inp[f"Wi_{nm}"]                      # [64, 384]
        bc = inp[f"b_emb_{nm}"] @ inp[f"Wi_{nm}"] + inp[f"bi_{nm}"]    # [384]
        for g in range(3):
            wobs[:OBS, 3 * h + g] = wc[:, g * H:(g + 1) * H]
            wobs[OBS, 3 * h + g] = bc[g * H:(g + 1) * H]
        whh[:, 3 * h + 0] = inp[f"Whrz_{nm}"][:, :H]
        whh[:, 3 * h + 1] = inp[f"Whrz_{nm}"][:, H:]
        whh[:, 3 * h + 2] = inp[f"Whn_{nm}"]
        bhn_l[0, h] = inp[f"bhn_{nm}"]
    bac = np.concatenate([inp["b_act"], inp["b_crit"]])[None, :]
    return {
        "wobs": _bf(wobs), "whh": _bf(whh), "bhn": _bf(bhn_l),
        "wth": _bf(inp["W_th"]), "wact": _bf(inp["W_act"]),
        "wcrit": _bf(inp["W_crit"]), "bth": _bf(inp["b_th"][None, :]),
        "bac": _bf(bac),
    }


def _host_exact_cols(inp, cols):
    """f32 reference recompute of the GRU stack for a subset of batch cols."""
    obs = inp["obs"][:, cols, :].astype(np.float32)        # [T, nb, OBS]
    h = {nm: inp["hidden"][cols, i * H:(i + 1) * H].astype(np.float32)
         for i, nm in enumerate(("t", "a", "c"))}
    d = inp["dones"][:, cols].astype(np.float32)           # [T, nb]
    ys = {}
    for nm in ("t", "a", "c"):
        emb = obs @ inp[f"W_emb_{nm}"] + inp[f"b_emb_{nm}"]
        xp = emb @ inp[f"Wi_{nm}"] + inp[f"bi_{nm}"]       # [T, nb, 3H]
        Whrz, Whn, bhn = inp[f"Whrz_{nm}"], inp[f"Whn_{nm}"], inp[f"bhn_{nm}"]
        hh = h[nm]
        y = np.zeros((T, len(cols), H), np.float32)
        for t in range(T):
            hh = hh * (1.0 - d[t])[:, None]
            hrz = hh @ Whrz
            r = 1.0 / (1.0 + np.exp(-(xp[t, :, :H] + hrz[:, :H])))
            z = 1.0 / (1.0 + np.exp(-(xp[t, :, H:2 * H] + hrz[:, H:])))
            n = np.tanh(xp[t, :, 2 * H:] + r * (hh @ Whn + bhn))
            hh = (1.0 - z) * n + z * hh
            y[t] = hh
        ys[nm] = y
        h[nm] = hh
    hid = np.concatenate([h["t"], h["a"], h["c"]], -1)
    act = ys["a"] @ inp["W_act"] + inp["b_act"]
    crit = (ys["c"] @ inp["W_crit"] + inp["b_crit"])[..., 0]
    th = 1.0 / (1.0 + np.exp(-(ys["t"] @ inp["W_th"] + inp["b_th"])))
    return hid, act, crit, th


# ---------------------------------------------------------------------------
# Entry
# ---------------------------------------------------------------------------
def _run(inputs, trace=False):
    import concourse.bass_utils as bass_utils
    if trace:
        import antenv
        from trn_agent_boot.trn_boot import _ntff_profile_via_ctypes
        if "antenv.axon_hooks" not in sys.modules:
            _m = types.ModuleType("antenv.axon_hooks")
            _h = _ntff_profile_via_ctypes('/opt/axon/libaxon_pjrt.so')
            _m.get_axon_ntff_profile_hook = lambda: _h
            _m.set_axon_ntff_profile_hook = lambda h: None
            sys.modules["antenv.axon_hooks"] = _m
            antenv.axon_hooks = _m
        bass_utils.upload_artifacts = lambda d: d

    inputs = {k: np.asarray(v) for k, v in inputs.items()}
    obs = inputs["obs"].astype(np.float32)
    dones = inputs["dones"].astype(bool)
    hidden = inputs["hidden"].astype(np.float32)

    S, W, badcols = _choose_sw(dones)
    if (S, W) not in _NC_CACHE:
        _NC_CACHE[(S, W)] = _build(S, W)
    nc = _NC_CACHE[(S, W)]

    weights = _prep_weights(inputs)
    in_maps = [_prep_core(c, S, W, obs, dones, hidden, weights)
               for c in range(NCORES)]
    res = bass_utils.run_bass_kernel_spmd(nc, in_maps, core_ids=list(range(NCORES)),
                                          trace=trace)

    hidden_out = np.zeros((B, 3 * H), np.float32)
    actor = np.zeros((T, B, A), np.float32)
    critic = np.zeros((T, B), np.float32)
    th = np.zeros((T, B, H), np.float32)
    for c in range(NCORES):
        b0 = c * BC
        r = res.results[c]
        hidden_out[b0:b0 + BC] = r["ylast"].reshape(H, 3, BC).transpose(2, 1, 0).reshape(BC, 3 * H)
        th[:, b0:b0 + BC, :] = r["th"]
        actor[:, b0:b0 + BC, :] = r["ac"][:, :, :A]
        critic[:, b0:b0 + BC] = r["ac"][:, :, A]

    if len(badcols):
        # exact host recompute for columns the segmentation cannot cover
        hid_p, act_p, crit_p, th_p = _host_exact_cols(inputs, badcols)
        hidden_out[badcols] = hid_p
        actor[:, badcols, :] = act_p
        critic[:, badcols] = crit_p
        th[:, badcols, :] = th_p
    return (hidden_out, actor, critic, th), res


def kernel(**inputs):
    out, _ = _run(inputs, trace=False)
    return out
